# revision 9
# baseline (speedup 1.0000x reference)
"""BinaryTreeCRF inside-algorithm kernel for 8 Trainium2 NeuronCores.

Strategy (hardcoded for hidden=[16383,1024], L=32, depth 13):
  - The 16383-node heap tree is cut at big-tree level 3: each of the 8 cores
    owns the 2047-node subtree rooted at heap node 7+c (big levels 3..13).
  - Per core, node hidden states are shipped transposed+bit-reversed so each
    tree level is a contiguous block of columns and left/right children are
    contiguous half-blocks (no strided access on device).
  - On device (bf16, label-on-partition layout):
      E^T = W @ hsT + b                                   (PE, 8 K-chunks)
      two combine levels (1024 leaves -> 512 -> 256 nodes) using the
      residual/accumulator decomposition  score = resid + acc:
        logP[(l,r), j] = (resid_l[l,j]-mean_l[j]) + (resid_r[r,j]-mean_r[j])
                         via selector matmuls with the mean folded in,
        P = exp(logP)  (ACT),  S^T = Texp @ P  (PE),
        resid' = E^T + ln S^T (ACT+DVE), acc' = acc_l+acc_r+mean_l+mean_r.
  - Host finishes the remaining small levels (256 -> subtree roots -> root)
    in float64 numpy; this is ~1% of the FLOPs.
"""

import numpy as np
import ml_dtypes

BF16 = ml_dtypes.bfloat16

INPUT_SIZE = 1024
L = 32
DEPTH = 13
N_CORES = 8
SUB_LEVELS = 11       # per-core subtree levels: 0 = 1024 leaves ... 10 = root
COLS = 2048           # per-core columns (2047 nodes + 1 zero pad)

# column offset of each subtree level in the per-core layout
OFFS = []
_o = 0
for _l in range(SUB_LEVELS):
    OFFS.append(_o)
    _o += 1 << (10 - _l)
assert _o == 2047


def _bitrev(x, bits):
    x = np.asarray(x, dtype=np.int64)
    out = np.zeros_like(x)
    for i in range(bits):
        out = (out << 1) | ((x >> i) & 1)
    return out


def _core_col_heap_index(c):
    """heap index for each of the 2047 real columns of core c's layout."""
    idx = np.zeros(2047, dtype=np.int64)
    for lev in range(SUB_LEVELS):
        m = 1 << (10 - lev)
        d = DEPTH - lev
        q = np.arange(m)
        j = _bitrev(q, 10 - lev)
        idx[OFFS[lev]: OFFS[lev] + m] = (1 << d) - 1 + c * m + j
    return idx


def _selectors():
    """Selector matrices (mean-subtraction folded in) for the logP matmuls."""
    selL = np.full((L, 8 * 128), -1.0 / L, dtype=np.float32)
    selR = np.full((L, 128), -1.0 / L, dtype=np.float32)
    for ch in range(8):
        for p in range(128):
            selL[ch * 4 + p // 32, ch * 128 + p] += 1.0
    for p in range(128):
        selR[p % 32, p] += 1.0
    return selL.astype(BF16), selR.astype(BF16)


_NC = None


def _build_bass():
    global _NC
    if _NC is not None:
        return _NC
    from concourse import bacc, mybir
    from concourse.tile import TileContext

    dtb = mybir.dt.bfloat16
    dtf = mybir.dt.float32
    AF = mybir.ActivationFunctionType

    nc = bacc.Bacc()
    hsT = nc.dram_tensor("hsT", [1024, COLS], dtb, kind="ExternalInput")
    # all constants in ONE tensor -> ONE DMA -> one HWDGE-lane dependency
    # (walrus caps per-instruction sync waits; scattered const DMAs blow it)
    cpk = nc.dram_tensor("cpack", [128, 1680], dtb, kind="ExternalInput")
    outE = nc.dram_tensor("outE", [L, COLS], dtb, kind="ExternalOutput")
    outResid = nc.dram_tensor("outResid", [L, 256], dtb, kind="ExternalOutput")
    outAcc = nc.dram_tensor("outAcc", [1, 256], dtf, kind="ExternalOutput")

    with TileContext(nc) as tc:
        with tc.tile_pool(name="consts", bufs=1) as consts, \
             tc.tile_pool(name="hs", bufs=8) as hpool, \
             tc.tile_pool(name="state", bufs=1) as state, \
             tc.tile_pool(name="pbuf", bufs=2) as pbuf, \
             tc.tile_pool(name="tmp", bufs=4) as tmp, \
             tc.tile_pool(name="ps2", bufs=3, space="PSUM") as ps2, \
             tc.tile_pool(name="smps", bufs=2, space="PSUM") as smps:

            cp = consts.tile([128, 1680], dtb, tag="cpack")
            nc.sync.dma_start(out=cp, in_=cpk[:, :])
            wTr_t = cp[:, 0:256]
            texp_t = cp[:, 256:512]
            selL_t = cp[0:L, 512:1536]
            selR_t = cp[0:L, 1536:1664]
            onesM_t = cp[0:L, 1664:1665]
            bias_t = cp[0:L, 1665:1666]

            # ACT warm-up: absorb the const-DMA wait into a cheap op so the
            # big E-copy activation needs only its PE wait (walrus allows a
            # single sync-wait on ACT instructions).
            warm = tmp.tile([L, 1], dtf, tag="warm")
            nc.scalar.activation(out=warm, in_=bias_t, func=AF.Identity)

            # ---- E^T = W @ hsT + b ----  (two column halves of 1024)
            E_bf = state.tile([L, COLS], dtb, tag="E_bf")
            hts = []
            for c in range(8):
                ht = hpool.tile([128, COLS], dtb, tag="hs")
                nc.sync.dma_start(out=ht, in_=hsT[c * 128:(c + 1) * 128, :])
                hts.append(ht)
            for h in range(2):
                psum_E = ps2.tile([L, 1024], dtf, tag="ps")
                for c in range(8):
                    for nb in range(2):
                        col = h * 1024 + nb * 512
                        nc.tensor.matmul(
                            psum_E[:, nb * 512:(nb + 1) * 512],
                            lhsT=wTr_t[:, c * L:(c + 1) * L],
                            rhs=hts[c][:, col:col + 512],
                            start=(c == 0), stop=(c == 7),
                        )
                nc.scalar.activation(out=E_bf[:, h * 1024:(h + 1) * 1024],
                                     in_=psum_E, func=AF.Identity,
                                     bias=bias_t, scale=1.0)
            nc.gpsimd.dma_start(out=outE[:, :], in_=E_bf)

            resid1 = state.tile([L, 512], dtb, tag="resid1")
            acc1 = state.tile([1, 512], dtf, tag="acc1")
            resid2 = state.tile([L, 256], dtb, tag="resid2")
            acc2 = state.tile([1, 256], dtf, tag="acc2")

            def combine_pass(rl, rr, elev, r_out, nj):
                """One combine sub-pass over nj parents; returns mean psum."""
                logP = ps2.tile([128, 8, nj], dtf, tag="ps")
                for c in range(8):
                    nc.tensor.matmul(
                        logP[:, c, :], lhsT=selL_t[:, c * 128:(c + 1) * 128],
                        rhs=rl, start=True, stop=False)
                    nc.tensor.matmul(
                        logP[:, c, :], lhsT=selR_t,
                        rhs=rr, start=False, stop=True)
                # meansum = mean_l + mean_r, summed on the PE via accumulation
                mean = smps.tile([1, nj], dtf, tag="small")
                nc.tensor.matmul(mean, lhsT=onesM_t, rhs=rl,
                                 start=True, stop=False)
                nc.tensor.matmul(mean, lhsT=onesM_t, rhs=rr,
                                 start=False, stop=True)
                P = pbuf.tile([128, 8, nj], dtb, tag="P")
                nc.scalar.activation(out=P, in_=logP, func=AF.Exp)
                S = smps.tile([L, nj], dtf, tag="small")
                for c in range(8):
                    nc.tensor.matmul(
                        S, lhsT=texp_t[:, c * L:(c + 1) * L], rhs=P[:, c, :],
                        start=(c == 0), stop=(c == 7))
                lnS = tmp.tile([L, nj], dtb, tag="lnS")
                nc.scalar.activation(out=lnS, in_=S, func=AF.Ln)
                nc.vector.tensor_add(r_out, lnS, elev)
                return mean

            # level 1: 1024 leaves -> 512, four sub-passes of 128 parents
            for j0 in (0, 128, 256, 384):
                mean = combine_pass(
                    rl=E_bf[:, j0:j0 + 128],
                    rr=E_bf[:, 512 + j0:512 + j0 + 128],
                    elev=E_bf[:, 1024 + j0:1024 + j0 + 128],
                    r_out=resid1[:, j0:j0 + 128], nj=128)
                nc.vector.tensor_copy(acc1[:, j0:j0 + 128], mean)

            # level 2: 512 -> 256, two sub-passes of 128 parents
            for j0 in (0, 128):
                mean = combine_pass(
                    rl=resid1[:, j0:j0 + 128], rr=resid1[:, 256 + j0:256 + j0 + 128],
                    elev=E_bf[:, 1536 + j0:1536 + j0 + 128],
                    r_out=resid2[:, j0:j0 + 128], nj=128)
                usum = tmp.tile([1, 128], dtf, tag="usum")
                nc.vector.tensor_add(usum, acc1[:, j0:j0 + 128],
                                     acc1[:, 256 + j0:256 + j0 + 128])
                nc.vector.tensor_add(acc2[:, j0:j0 + 128], usum, mean)

            nc.gpsimd.dma_start(out=outResid[:, :], in_=resid2)
            nc.gpsimd.dma_start(out=outAcc[:, :], in_=acc2)

    nc.compile()
    _NC = nc
    return nc


def _prep_in_maps(hidden, W, b, trans):
    """Build per-core input dicts (host-side shard/transpose/cast)."""
    wTr = np.ascontiguousarray(
        W.T.reshape(8, 128, L).transpose(1, 0, 2).reshape(128, 8 * L)
    ).astype(BF16)
    texpT = np.exp(trans.astype(np.float64)).astype(np.float32)  # [k, l, r]
    texpT = texpT.transpose(1, 2, 0).reshape(L * L, L)           # [(l r), k]
    texpTr = np.ascontiguousarray(
        texpT.reshape(8, 128, L).transpose(1, 0, 2).reshape(128, 8 * L)
    ).astype(BF16)
    selL, selR = _selectors()

    cpack = np.zeros((128, 1680), dtype=BF16)
    cpack[:, 0:256] = wTr
    cpack[:, 256:512] = texpTr
    cpack[0:L, 512:1536] = selL
    cpack[0:L, 1536:1664] = selR
    cpack[0:L, 1664] = BF16(1.0 / L)
    cpack[0:L, 1665] = b.astype(BF16)

    in_maps = []
    for c in range(N_CORES):
        idx = _core_col_heap_index(c)
        rows = hidden[idx].astype(BF16)              # [2047, 1024]
        hsT = np.zeros((1024, COLS), dtype=BF16)
        hsT[:, :2047] = rows.T
        in_maps.append({
            "hsT": np.ascontiguousarray(hsT),
            "cpack": cpack,
        })
    return in_maps


def _host_finish(results, hidden, W, b, trans):
    """Finish levels 3..10 per core + big-tree top 3 levels, in float64."""
    Texp = np.exp(trans.astype(np.float64)).reshape(L, L * L)   # [k, (l r)]

    # per-core level-2 scores (256 nodes each), un-bit-reversed
    score = np.zeros((N_CORES, 256, L))
    elev_nat = {}   # (core, lev) -> [m, L] natural-order E
    for c in range(N_CORES):
        r = results[c]
        E = r["outE"].astype(np.float64)            # [L, 2048]
        resid2 = r["outResid"].astype(np.float64)   # [L, 256]
        acc2 = r["outAcc"].astype(np.float64)       # [1, 256]
        q = _bitrev(np.arange(256), 8)
        score[c] = (resid2 + acc2)[:, q].T          # node j at col bitrev(j)
        for lev in range(3, SUB_LEVELS):
            m = 1 << (10 - lev)
            qq = _bitrev(np.arange(m), 10 - lev)
            elev_nat[(c, lev)] = E[:, OFFS[lev] + qq].T

    # subtree levels 3..10 (vectorized over cores)
    for lev in range(3, SUB_LEVELS):
        left = score[:, 0::2]                      # [8, m, L]
        right = score[:, 1::2]
        Elev = np.stack([elev_nat[(c, lev)] for c in range(N_CORES)])
        ml = left.max(axis=2, keepdims=True)
        mr = right.max(axis=2, keepdims=True)
        P = (np.exp(left - ml)[..., :, None] *
             np.exp(right - mr)[..., None, :]).reshape(N_CORES, -1, L * L)
        score = Elev + np.log(P @ Texp.T) + ml + mr

    # big-tree top: level-3 scores are the 8 subtree roots, heap nodes 7..14
    score = score.reshape(8, L)                     # [8 nodes, L]
    Etop = (hidden[:7].astype(np.float64) @ W.astype(np.float64).T
            + b.astype(np.float64))                 # heap nodes 0..6
    for d in (2, 1, 0):
        left = score[0::2]
        right = score[1::2]
        Elev = Etop[(1 << d) - 1: (1 << (d + 1)) - 1]
        ml = left.max(axis=1, keepdims=True)
        mr = right.max(axis=1, keepdims=True)
        P = (np.exp(left - ml)[:, :, None] *
             np.exp(right - mr)[:, None, :]).reshape(-1, L * L)
        score = Elev + np.log(P @ Texp.T) + ml + mr
    return score[0].astype(np.float32)


def _run_spmd(in_maps, trace=False):
    from concourse.bass_utils import run_bass_kernel_spmd
    nc = _build_bass()
    return run_bass_kernel_spmd(nc, in_maps, list(range(N_CORES)), trace=trace)


def kernel(hidden, W, b, trans):
    hidden = np.asarray(hidden, dtype=np.float32)
    W = np.asarray(W, dtype=np.float32)
    b = np.asarray(b, dtype=np.float32)
    trans = np.asarray(trans, dtype=np.float32)
    in_maps = _prep_in_maps(hidden, W, b, trans)
    res = _run_spmd(in_maps, trace=False)
    return _host_finish(res.results, hidden, W, b, trans)
